# revision 23
# baseline (speedup 1.0000x reference)
"""MultiHeadCrossAttention on 8 Trainium2 cores.

Sharding: core c -> (batch b = c//2, head-group g = c%2). Each core computes
q/k/v projections for its 8 heads (feature shard of 512), full attention for
those heads over its batch, and a partial output projection; the host sums
the two partial outputs per batch (tensor-parallel all-reduce done host-side).

Device layout is feature-on-partition ("transposed") throughout:
  kT [512, 2048] bf16, q slices [128, 512], v [2048, 768] bf16 augmented
  with shared ones columns, scoresT [m, n] per head.

The kernel is one software-pipelined instruction stream built from chunk
closures: per (nb, hp) unit, the scores matmuls (K=64, even/odd heads on
distinct PE row groups so they can run concurrently), the exp stream on the
Activation engine, the attn@V matmuls of the unit two steps back, the next
unit's q projection, and the output projection are interleaved so no engine
waits on another's batch. x tensors are staged in slabs (xq per query
block, xv per quarter) to fit SBUF. Softmax sums ride along in attn@V via
the shared ones columns; 1/sum via DVE reciprocal; partition broadcast via
gpsimd (even head, sums at partition 0) or a K=1 PE outer product (odd).
"""

import numpy as np
import ml_dtypes

B, N, M, EMBED, HEADS, D = 4, 2048, 2048, 1024, 16, 64
F = 512          # features per core = 8 heads
NCORES = 8

_NC = None


def _build_nc(repeat=1):
    from contextlib import ExitStack
    import concourse.bass as bass  # noqa: F401
    import concourse.tile as tile
    import concourse.mybir as mybir
    from concourse import bacc

    f32 = mybir.dt.float32
    bf16 = mybir.dt.bfloat16
    AF = mybir.ActivationFunctionType

    nc = bacc.Bacc(None, target_bir_lowering=False)

    xqT = nc.dram_tensor("xqT", [EMBED, N], bf16, kind="ExternalInput")
    xkT = nc.dram_tensor("xkT", [EMBED, M], bf16, kind="ExternalInput")
    xvT = nc.dram_tensor("xvT", [EMBED, M], bf16, kind="ExternalInput")
    wqT = nc.dram_tensor("wqT", [EMBED, F], bf16, kind="ExternalInput")
    wkT = nc.dram_tensor("wkT", [EMBED, F], bf16, kind="ExternalInput")
    wvT = nc.dram_tensor("wvT", [EMBED, F], bf16, kind="ExternalInput")
    woT = nc.dram_tensor("woT", [F, EMBED], bf16, kind="ExternalInput")
    out_d = nc.dram_tensor("out", [N, EMBED], bf16, kind="ExternalOutput")

    EC = EMBED // 128   # 8 contraction chunks for projections
    FB = F // 128       # 4 feature chunks = head pairs
    NB = N // 512       # 4 query blocks
    MC = M // 128       # 16 key chunks
    VW = 192            # v_aug cols per head pair: [v_odd | ones | v_even]
    scale = 1.0 / np.sqrt(D)

    def run(chunks):
        for c in chunks:
            c()

    def rr(*lists):
        """Round-robin execute chunk closures from several lists."""
        lists = [list(x) for x in lists]
        while any(lists):
            for x in lists:
                if x:
                    x.pop(0)()

    with tile.TileContext(nc) as tc:
        with ExitStack() as top:
            persist = top.enter_context(tc.tile_pool(name="persist", bufs=1))
            kT = [persist.tile([128, M], bf16, name=f"kT{i}", tag=f"kT{i}")
                  for i in range(FB)]
            v_sb = [persist.tile([128, FB * VW], bf16, name=f"v{i}", tag=f"v{i}")
                    for i in range(MC)]
            wo_sb = [persist.tile([128, EMBED], bf16, name=f"wo{i}", tag=f"wo{i}")
                     for i in range(4)]
            ones_f = persist.tile([128, 64], f32, tag="ones_f")
            nc.vector.memset(ones_f, 1.0)
            # double-buffered zero-padded q tiles; the zero halves are
            # written once and never touched again
            qe_t = [persist.tile([128, 512], bf16, name=f"qe{i}", tag=f"qe{i}")
                    for i in range(2)]
            qo_t = [persist.tile([128, 512], bf16, name=f"qo{i}", tag=f"qo{i}")
                    for i in range(2)]
            for i in range(2):
                nc.vector.memset(qe_t[i][64:128, :], 0.0)
                nc.vector.memset(qo_t[i][0:64, :], 0.0)
            # shared ones columns of v_aug, written once (the per-rep v
            # copies never touch them)
            for mc in range(MC):
                for hp in range(FB):
                    nc.gpsimd.memset(
                        v_sb[mc][:, hp * VW + 64:hp * VW + 128], 1.0)

            for _rep in range(repeat):
                with tc.tile_pool(name="lda", bufs=1) as lda, \
                     tc.tile_pool(name="sl", bufs=2) as sl, \
                     tc.tile_pool(name="atp", bufs=2) as atp, \
                     tc.tile_pool(name="expp", bufs=3) as expp, \
                     tc.tile_pool(name="rcp", bufs=2) as rcp, \
                     tc.tile_pool(name="ost", bufs=2) as ost, \
                     tc.tile_pool(name="ps_sc", bufs=2, space="PSUM") as ps_sc, \
                     tc.tile_pool(name="ps_at", bufs=2, space="PSUM") as ps_at, \
                     tc.tile_pool(name="ps_ms", bufs=2, space="PSUM") as ps_ms:

                    # ---- staged input DMAs ----
                    def dma_w(w_dram, wtag):
                        w_sb = [lda.tile([128, F], bf16, name=f"{wtag}{e}",
                                         tag=f"{wtag}{e}") for e in range(EC)]
                        for e in range(EC):
                            nc.sync.dma_start(
                                out=w_sb[e], in_=w_dram[e * 128:(e + 1) * 128, :])
                        return w_sb

                    def alloc_xk():
                        return [lda.tile([128, M], bf16, name=f"ax{e}",
                                         tag=f"ax{e}") for e in range(EC)]

                    def dma_xk_nb(x_sb, nb):
                        csl = slice(nb * 512, (nb + 1) * 512)
                        for e in range(EC):
                            nc.sync.dma_start(
                                out=x_sb[e][:, csl],
                                in_=xkT[e * 128:(e + 1) * 128, csl])

                    def dma_slab(x_dram, tag, csl):
                        """Stage 512 columns of x (all EC chunks)."""
                        x_sb = [sl.tile([128, 512], bf16, name=f"{tag}{e}",
                                        tag=f"{tag}{e}") for e in range(EC)]
                        for e in range(EC):
                            nc.sync.dma_start(
                                out=x_sb[e], in_=x_dram[e * 128:(e + 1) * 128, csl])
                        return x_sb

                    xq_slabs = {}
                    xv_slabs = {}

                    def dma_xq(nb):
                        xq_slabs[nb] = dma_slab(
                            xqT, "bx", slice(nb * 512, (nb + 1) * 512))

                    def dma_xv(qtr):
                        xv_slabs[qtr] = dma_slab(
                            xvT, "vx", slice(qtr * 512, (qtr + 1) * 512))

                    # ---- chunk builders (each returns a list of closures) ----
                    def k_chunks(hp):
                        def grp(nb):
                            def f():
                                ps = ps_ms.tile([128, 512], f32, name="pk", tag="ms")
                                for e in range(EC):
                                    nc.tensor.matmul(
                                        ps, lhsT=wk_sb[e][:, hp * 128:(hp + 1) * 128],
                                        rhs=xk_sb[e][:, nb * 512:(nb + 1) * 512],
                                        start=(e == 0), stop=(e == EC - 1))
                                nc.vector.tensor_copy(
                                    kT[hp][:, nb * 512:(nb + 1) * 512], ps)
                            return f
                        return [grp(nb) for nb in range(NB)]

                    q_tiles = {}

                    def q_chunks(nb, hp):
                        state = {}

                        def f1():
                            u = 4 * nb + hp
                            q_tiles[(nb, hp)] = (qe_t[u % 2], qo_t[u % 2])
                            ps = ps_ms.tile([128, 512], f32, name="pq", tag="ms")
                            state["ps"] = ps
                            xs = xq_slabs[nb]
                            for e in range(4):
                                nc.tensor.matmul(
                                    ps, lhsT=wq_sb[e][:, hp * 128:(hp + 1) * 128],
                                    rhs=xs[e], start=(e == 0), stop=False)

                        def f2():
                            ps = state["ps"]
                            xs = xq_slabs[nb]
                            for e in range(4, EC):
                                nc.tensor.matmul(
                                    ps, lhsT=wq_sb[e][:, hp * 128:(hp + 1) * 128],
                                    rhs=xs[e], start=False, stop=(e == EC - 1))
                            qe, qo = q_tiles[(nb, hp)]
                            # only the live halves are rewritten; zero halves
                            # are static
                            nc.vector.tensor_copy(qe[0:64, :], ps[0:64, :])
                            nc.vector.tensor_copy(qo[64:128, :], ps[64:128, :])

                        return [f1, f2]

                    def v_chunk(mb):
                        def f():
                            xs = xv_slabs[mb // 4]
                            c0 = (mb % 4) * 128
                            ps = ps_ms.tile([128, 512], f32, name="pv", tag="ms")
                            for e in range(EC):
                                nc.tensor.matmul(
                                    ps, lhsT=xs[e][:, c0:c0 + 128],
                                    rhs=wv_sb[e][:, 0:F],
                                    start=(e == 0), stop=(e == EC - 1))
                            # psum cols per head pair: [even(64) | odd(64)]
                            # -> v_aug cols [odd | ones | even]
                            vv = v_sb[mb].rearrange("p (a b) -> p a b", b=VW)
                            pv = ps.rearrange("p (a b) -> p a b", b=128)
                            nc.vector.tensor_copy(vv[:, :, 128:192], pv[:, :, 0:64])
                            nc.vector.tensor_copy(vv[:, :, 0:64], pv[:, :, 64:128])
                        return f

                    exp_tiles = {}

                    def sc_chunks(nb, hp):
                        # exp outputs go to half-unit tiles (8 m-chunks each)
                        # so the 6-slot pool recycles at half-unit
                        # granularity: the slot a tile reuses is always freed
                        # by attn@V chunks emitted in an earlier round.
                        def mk(mcp):
                            def f():
                                if mcp == 0:
                                    exp_tiles[(nb, hp)] = {"q": q_tiles.pop((nb, hp))}
                                tl = exp_tiles[(nb, hp)]
                                if mcp % 4 == 0:
                                    h = mcp // 4
                                    tl["e", h] = expp.tile(
                                        [128, 4096], bf16, name="expe", tag="exp")
                                    tl["o", h] = expp.tile(
                                        [128, 4096], bf16, name="expo", tag="exp")
                                qe, qo = tl["q"]
                                # alternate allocation order so both tiles'
                                # slot-recycle waits resolve on the same exp
                                # event and the matmul burst issues
                                # back-to-back
                                if mcp % 2:
                                    pso = ps_sc.tile([128, 1024], f32, name="pso", tag="sc")
                                    pse = ps_sc.tile([128, 1024], f32, name="pse", tag="sc")
                                else:
                                    pse = ps_sc.tile([128, 1024], f32, name="pse", tag="sc")
                                    pso = ps_sc.tile([128, 1024], f32, name="pso", tag="sc")
                                for j in (0, 1):
                                    mc = 2 * mcp + j
                                    msl = slice(mc * 128, (mc + 1) * 128)
                                    jsl = slice(j * 512, (j + 1) * 512)
                                    nc.tensor.matmul(
                                        pse[:, jsl], lhsT=kT[hp][:, msl],
                                        rhs=qe, start=True, stop=True)
                                    nc.tensor.matmul(
                                        pso[:, jsl], lhsT=kT[hp][:, msl],
                                        rhs=qo, start=True, stop=True)
                                esl = slice((mcp % 4) * 1024, (mcp % 4 + 1) * 1024)
                                nc.scalar.activation(tl["e", mcp // 4][:, esl],
                                                     pse[:, :], AF.Exp, scale=scale)
                                nc.scalar.activation(tl["o", mcp // 4][:, esl],
                                                     pso[:, :], AF.Exp, scale=scale)
                            return f
                        return [mk(m) for m in range(MC // 2)]

                    at_tiles = {}
                    av_state = {}

                    def av_chunks(nb, hp):
                        v0 = hp * VW

                        def mk(mcp):
                            def f():
                                if mcp == 0:
                                    att = atp.tile([128, 512], bf16, name="att",
                                                   tag=f"at{hp}")
                                    at_tiles[(nb, hp)] = att
                                    av_state[(nb, hp)] = (
                                        ps_at.tile([128, 512], f32, name="psae", tag="at"),
                                        ps_at.tile([128, 512], f32, name="psao", tag="at"),
                                    )
                                tl = exp_tiles[(nb, hp)]
                                psae, psao = av_state[(nb, hp)]
                                for j in (0, 1):
                                    mc = 2 * mcp + j
                                    ksl = slice((mc % 8) * 512, (mc % 8 + 1) * 512)
                                    st, sp = (mc == 0), (mc == MC - 1)
                                    nc.tensor.matmul(
                                        psae, lhsT=v_sb[mc][:, v0 + 64:v0 + 192],
                                        rhs=tl["e", mc // 8][:, ksl], start=st, stop=sp)
                                    nc.tensor.matmul(
                                        psao, lhsT=v_sb[mc][:, v0:v0 + 128],
                                        rhs=tl["o", mc // 8][:, ksl], start=st, stop=sp)
                            return f

                        def norm_e():
                            # even head: sums at partition 0 -> direct
                            # reciprocal from PSUM + gpsimd broadcast
                            psae, _ = av_state[(nb, hp)]
                            att = at_tiles[(nb, hp)]
                            srow = rcp.tile([128, 1024], f32, name="srow", tag="sr", bufs=1)
                            aux_e = rcp.tile([128, 512], f32, name="auxe", tag="aux")
                            av_state[(nb, hp, "sr")] = srow
                            nc.vector.reciprocal_approx_accurate(
                                out=srow[0:1, 0:512], in_=psae[0:1, :],
                                scratch=srow[0:1, 512:1024])
                            nc.gpsimd.partition_broadcast(
                                aux_e[:, :], srow[0:1, 0:512])
                            nc.vector.tensor_mul(att[64:128, :],
                                                 psae[64:128, :], aux_e[64:128, :])

                        def norm_o():
                            # odd head: sums at partition 64 -> copy to SBUF,
                            # K=1 PE outer product down to partitions 0:64,
                            # reciprocal at base 0
                            _, psao = av_state[(nb, hp)]
                            att = at_tiles[(nb, hp)]
                            srow = av_state.pop((nb, hp, "sr"))
                            del av_state[(nb, hp)]
                            aux_o = rcp.tile([128, 512], f32, name="auxo", tag="aux")
                            nc.vector.tensor_copy(
                                srow[64:65, 512:1024], psao[64:65, :])
                            psr2 = ps_ms.tile([128, 512], f32, name="psr2", tag="ms")
                            nc.tensor.matmul(
                                psr2[0:64, :], lhsT=ones_f[64:65, 0:64],
                                rhs=srow[64:65, 512:1024], start=True, stop=True)
                            nc.vector.reciprocal_approx_accurate(
                                out=aux_o[0:64, :], in_=psr2[0:64, :],
                                scratch=srow[0:64, 0:512])
                            nc.vector.tensor_mul(att[0:64, :],
                                                 psao[0:64, :], aux_o[0:64, :])
                            del exp_tiles[(nb, hp)]

                        return [mk(m) for m in range(MC // 2)] + [norm_e, norm_o]

                    def oproj_chunks(nb):
                        def mk(nch, ob):
                            def f():
                                n0 = nb * 512 + nch * 128
                                csl = slice(nch * 128, (nch + 1) * 128)
                                ps = ps_ms.tile([128, 512], f32, name="po", tag="ms")
                                for fc in range(FB):
                                    nc.tensor.matmul(
                                        ps, lhsT=at_tiles[(nb, fc)][:, csl],
                                        rhs=wo_sb[fc][:, ob * 512:(ob + 1) * 512],
                                        start=(fc == 0), stop=(fc == FB - 1))
                                o_sb = ost.tile([128, 512], bf16, name="o", tag="o")
                                nc.vector.tensor_copy(o_sb, ps)
                                nc.sync.dma_start(
                                    out=out_d[n0:n0 + 128,
                                              ob * 512:(ob + 1) * 512],
                                    in_=o_sb)
                            return f
                        return [mk(nch, ob) for nch in range(4) for ob in range(2)]

                    # ---- emission schedule ----
                    U = [(nb, hp) for nb in range(NB) for hp in range(FB)]

                    wk_sb = dma_w(wkT, "aw")
                    xk_sb = alloc_xk()
                    dma_xk_nb(xk_sb, 0)
                    dma_xk_nb(xk_sb, 1)
                    wq_sb = dma_w(wqT, "bw")
                    dma_xq(0)
                    dma_xk_nb(xk_sb, 2)
                    dma_xk_nb(xk_sb, 3)
                    wv_sb = dma_w(wvT, "wv")
                    for qtr in range(4):
                        dma_xv(qtr)
                    dma_xq(1)
                    for i in range(4):
                        nc.sync.dma_start(out=wo_sb[i],
                                          in_=woT[i * 128:(i + 1) * 128, :])

                    V = [v_chunk(mb) for mb in range(MC)]
                    pad = lambda: None  # noqa: E731
                    k0 = k_chunks(0)
                    # merged prelude: scores(U0) start as soon as k0.g0 and
                    # q(U0) have landed; k/q/v projections fill the
                    # DMA-paced front
                    rr([k0[0]] + q_chunks(*U[0]) + k0[1:4] + k_chunks(1)
                       + q_chunks(*U[1]) + V[0:8],
                       [pad, pad, pad] + sc_chunks(*U[0]))
                    rr(V[8:16] + k_chunks(2) + q_chunks(*U[2]),
                       av_chunks(*U[0]),
                       sc_chunks(*U[1]))
                    rr(av_chunks(*U[1]), k_chunks(3) + q_chunks(*U[3]),
                       sc_chunks(*U[2]))
                    for i in range(3, 16):
                        nb, hp = U[i]
                        extras = []
                        if i < 15:
                            extras += q_chunks(*U[i + 1])
                        if i == 5:
                            dma_xq(2)
                        if i == 9:
                            dma_xq(3)
                        if i in (5, 9, 13):
                            extras += oproj_chunks((i - 5) // 4)
                        rr(av_chunks(*U[i - 1]), extras, sc_chunks(*U[i]))
                    run(av_chunks(*U[15]))
                    run(oproj_chunks(3))

    nc.compile()
    return nc


def _get_nc(repeat=1):
    global _NC
    if repeat != 1:
        return _build_nc(repeat)
    if _NC is None:
        _NC = _build_nc()
    return _NC


def _make_in_maps(inputs):
    bf = ml_dtypes.bfloat16
    xq = np.asarray(inputs["xq"], np.float32)
    xk = np.asarray(inputs["xk"], np.float32)
    xv = np.asarray(inputs["xv"], np.float32)
    Wq = np.asarray(inputs["Wq"], np.float32)
    Wk = np.asarray(inputs["Wk"], np.float32)
    Wv = np.asarray(inputs["Wv"], np.float32)
    Wo = np.asarray(inputs["Wo"], np.float32)

    xqT = [np.ascontiguousarray(xq[b].T).astype(bf) for b in range(B)]
    xkT = [np.ascontiguousarray(xk[b].T).astype(bf) for b in range(B)]
    xvT = [np.ascontiguousarray(xv[b].T).astype(bf) for b in range(B)]
    wqT = [np.ascontiguousarray(Wq[g * F:(g + 1) * F, :].T).astype(bf) for g in range(2)]
    wkT = [np.ascontiguousarray(Wk[g * F:(g + 1) * F, :].T).astype(bf) for g in range(2)]
    wvT = [np.ascontiguousarray(Wv[g * F:(g + 1) * F, :].T).astype(bf) for g in range(2)]
    # attnT rows per 128-chunk are [odd-head dims | even-head dims]; reorder
    # Wo input rows to match
    woT = []
    for g in range(2):
        w = np.ascontiguousarray(Wo[:, g * F:(g + 1) * F].T).astype(bf)
        w = np.ascontiguousarray(
            w.reshape(4, 2, 64, EMBED)[:, ::-1].reshape(F, EMBED))
        woT.append(w)

    in_maps = []
    for c in range(NCORES):
        b, g = divmod(c, 2)
        in_maps.append({
            "xqT": xqT[b], "xkT": xkT[b], "xvT": xvT[b],
            "wqT": wqT[g], "wkT": wkT[g], "wvT": wvT[g], "woT": woT[g],
        })
    return in_maps


def kernel(**inputs) -> np.ndarray:
    from concourse.bass_utils import run_bass_kernel_spmd

    nc = _get_nc()
    in_maps = _make_in_maps(inputs)
    res = run_bass_kernel_spmd(nc, in_maps, core_ids=list(range(NCORES))).results
    out = np.empty((B, N, EMBED), np.float32)
    for b in range(B):
        out[b] = (res[2 * b]["out"].astype(np.float32)
                  + res[2 * b + 1]["out"].astype(np.float32))
    return out
